# revision 15
# baseline (speedup 1.0000x reference)
"""AbsorbingGraphKernel for trn2: data-parallel over batch (4 graphs/core x 8 cores).

Per graph:
  nodes: top-k_n absorbed gumbel -> unmask -> argmax(logits_x+g_x) -> one-hot
  edges: top-k_e absorbed gumbel on triu -> unmask -> argmax(logits_e+g_e)
         -> symmetrize -> one-hot with zeroed diagonal
Exact top-k threshold found by float bisection over per-chunk top-8 candidates.
"""
import numpy as np

import concourse.bass as bass
import concourse.mybir as mybir
import concourse.tile as tile
from concourse.bass_utils import run_bass_kernel_spmd

# this walrus build cannot codegen the EventSemaphore butterfly barrier nor
# a Drain carrying sem waits; NRT drains DMA at execution end, so skip both.
tile.TileContext._drain_and_barrier = lambda self, tick_clock, wait_clock: None

F32 = mybir.dt.float32
I32 = mybir.dt.int32
I8 = mybir.dt.int8
AL = mybir.AluOpType
AX = mybir.AxisListType

GPB = 4          # graphs per core
N = 512
C6 = 6
NEG = -1.0e9
BISECT_ITERS = 30


def _argmax6(nc, vpool, src3, P, width, out_pv, tagp, iota6bc):
    """First-index argmax over last dim of src3 [P,width,6] -> out_pv [P,width] f32.
    idx = 1000 + min_c(-1000*[v==max] + c)."""
    mx = vpool.tile([P, width], F32, name="amx0", tag=f"amx0{tagp}")
    nc.vector.tensor_reduce(mx[:], src3, axis=AX.X, op=AL.max)
    eq = vpool.tile([P, width * 6], F32, name="amx1", tag=f"amx1{tagp}")
    mxb = mx[:].rearrange("p (w o) -> p w o", o=1).broadcast_to([P, width, 6])
    eq3 = eq[:].rearrange("p (w c) -> p w c", c=6)
    nc.vector.tensor_tensor(eq3, src3, mxb, op=AL.is_equal)
    tmp = vpool.tile([P, width * 6], F32, name="amx2", tag=f"amx2{tagp}")
    nc.vector.scalar_tensor_tensor(out=tmp[:].rearrange("p (w c) -> p w c", c=6),
                                   in0=eq3, scalar=-1000.0,
                                   in1=iota6bc, op0=AL.mult, op1=AL.add)
    nc.vector.tensor_reduce(out_pv, tmp[:].rearrange("p (w c) -> p w c", c=6),
                            axis=AX.X, op=AL.min)
    nc.vector.tensor_scalar_add(out_pv, out_pv, 1000.0)


def _split_multi_waits(nc):
    """This walrus codegen accepts at most ONE sem wait per instruction.
    Tile emits joins with several; peel extras onto EventSemaphore carriers
    inserted immediately before, on the same engine (same program point, so
    semantics are identical)."""
    import copy
    tmpl = None
    for b in nc.m.functions[0].blocks:
        for i in b.instructions:
            if str(i.opcode) == "EventSemaphore" and i.sync_info is not None:
                tmpl = i
                break
        if tmpl is not None:
            break
    assert tmpl is not None, "no EventSemaphore template found"
    cnt = 0
    for f in nc.m.functions:
        for b in f.blocks:
            out = []
            changed = False
            for i in b.instructions:
                si = i.sync_info
                if si is not None and len(si.on_wait) > 1:
                    waits = [copy.deepcopy(w) for w in si.on_wait]
                    for w in waits[:-1]:
                        cnt += 1
                        c = copy.deepcopy(tmpl)
                        c.name = f"waitc_{cnt}"
                        c.engine = i.engine
                        csi = c.sync_info
                        csi.on_wait = [w]
                        csi.on_update = []
                        c.sync_info = csi
                        out.append(c)
                    si.on_wait = [waits[-1]]
                    i.sync_info = si
                    changed = True
                out.append(i)
            if changed:
                b.instructions = out


def build_kernel():
    nc = bass.Bass()
    ecl = nc.declare_dram_parameter("eclass", [GPB, N, N], I32, isOutput=False)
    ged = nc.declare_dram_parameter("gedge", [GPB, N, N], F32, isOutput=False)
    les = nc.declare_dram_parameter("logitse", [GPB, N, N, C6], F32, isOutput=False)
    ge6 = nc.declare_dram_parameter("ge6", [GPB, N, N, C6], F32, isOutput=False)
    lxs = nc.declare_dram_parameter("logitsx", [GPB, N, C6], F32, isOutput=False)
    gx6 = nc.declare_dram_parameter("gx6", [GPB, N, C6], F32, isOutput=False)
    xcl = nc.declare_dram_parameter("xclass", [GPB, N], I32, isOutput=False)
    gnd = nc.declare_dram_parameter("gnode", [GPB, N], F32, isOutput=False)
    ste = nc.declare_dram_parameter("stepe", [128, 1], F32, isOutput=False)
    stn = nc.declare_dram_parameter("stepn", [128, 1], F32, isOutput=False)
    enew = nc.declare_dram_parameter("enew", [GPB, N, N, C6], F32, isOutput=True)
    xnew = nc.declare_dram_parameter("xnew", [GPB, N, C6], F32, isOutput=True)

    with tile.TileContext(nc) as tc:
        with tc.tile_pool(name="const", bufs=1) as cp:
            ident = cp.tile([128, 128], F32)
            nc.vector.memset(ident[:], 1.0)
            nc.gpsimd.affine_select(ident[:], ident[:], pattern=[[1, 128]],
                                    compare_op=AL.is_equal, fill=0.0,
                                    base=0, channel_multiplier=-1)
            ones = cp.tile([128, 128], F32)
            nc.vector.memset(ones[:], 1.0)
            iota6i = cp.tile([128, 6], I32)
            nc.gpsimd.iota(iota6i[:], pattern=[[1, 6]], base=0, channel_multiplier=0)
            iota6f = cp.tile([128, 6], F32)
            nc.vector.tensor_copy(iota6f[:], iota6i[:])
            iota8i = cp.tile([128, 8], I32)
            nc.gpsimd.iota(iota8i[:], pattern=[[1, 8]], base=0, channel_multiplier=0)
            iota8f = cp.tile([128, 8], F32)
            nc.vector.tensor_copy(iota8f[:], iota8i[:])
            dmask6 = cp.tile([128, 4 * N], F32)
            nc.vector.memset(dmask6[:], 6.0)
            nc.gpsimd.affine_select(
                dmask6[:].rearrange("p (s j) -> p s j", s=4),
                dmask6[:].rearrange("p (s j) -> p s j", s=4),
                pattern=[[-128, 4], [1, N]], compare_op=AL.is_equal, fill=0.0,
                base=0, channel_multiplier=-1)
            ke_bc = cp.tile([128, 1], F32)
            nc.sync.dma_start(out=ke_bc[:], in_=ste[:])
            kn_bc = cp.tile([128, 1], F32)
            nc.sync.dma_start(out=kn_bc[:], in_=stn[:])

            # ===================== nodes (separate scope) =====================
            with tc.tile_pool(name="nodes", bufs=1) as np_:
                lxn = np_.tile([GPB, N * C6], F32)
                nc.sync.dma_start(out=lxn[:], in_=lxs[:].rearrange("g n c -> g (n c)"))
                gxn = np_.tile([GPB, N * C6], F32)
                nc.sync.dma_start(out=gxn[:], in_=gx6[:].rearrange("g n c -> g (n c)"))
                xcn = np_.tile([GPB, N], I32)
                nc.sync.dma_start(out=xcn[:], in_=xcl[:])
                gnn = np_.tile([GPB, N], F32)
                nc.sync.dma_start(out=gnn[:], in_=gnd[:])

                keyn = np_.tile([GPB, N], F32)
                cntn = np_.tile([GPB, 1], F32)
                nscr = np_.tile([GPB, N], F32)
                nc.vector.tensor_scalar(out=nscr[:], in0=xcn[:], scalar1=5.0,
                                        scalar2=None, op0=AL.is_equal)
                nc.vector.tensor_reduce(cntn[:], nscr[:], axis=AX.X, op=AL.add)
                nc.vector.tensor_mul(keyn[:], nscr[:], gnn[:])
                top8 = np_.tile([GPB, 8], F32)
                nc.vector.max(top8[:], keyn[:])
                kn = np_.tile([GPB, 1], F32)
                nc.vector.tensor_scalar(out=kn[:], in0=cntn[:], scalar1=kn_bc[0:GPB, :],
                                        scalar2=None, op0=AL.min)
                knm1 = np_.tile([GPB, 1], F32)
                nc.vector.tensor_scalar(out=knm1[:], in0=kn[:], scalar1=1.0,
                                        scalar2=None, op0=AL.subtract)
                eqk = np_.tile([GPB, 8], F32)
                nc.vector.tensor_scalar(out=eqk[:], in0=iota8f[0:GPB, :], scalar1=knm1[:],
                                        scalar2=None, op0=AL.is_equal)
                thn8 = np_.tile([GPB, 8], F32)
                nc.vector.tensor_mul(thn8[:], top8[:], eqk[:])
                thn = np_.tile([GPB, 1], F32)
                nc.vector.tensor_reduce(thn[:], thn8[:], axis=AX.X, op=AL.add)
                seln = np_.tile([GPB, N], I8)
                nc.vector.tensor_scalar(out=seln[:], in0=keyn[:], scalar1=thn[:],
                                        scalar2=None, op0=AL.is_ge)

                nc.vector.tensor_add(lxn[:], lxn[:], gxn[:])
                pvn = np_.tile([GPB, N], F32)
                i6nb = (iota6f[0:GPB, :].rearrange("g (o c) -> g o c", o=1)
                        .broadcast_to([GPB, N, 6]))
                _argmax6(nc, np_, lxn[:].rearrange("g (n c) -> g n c", c=6),
                         GPB, N, pvn[:], "n", i6nb)
                xoh = np_.tile([GPB, N * C6], F32)
                xoh3 = xoh[:].rearrange("g (n c) -> g n c", c=6)
                xcn_r = xcn[:].rearrange("g (n o) -> g n o", o=1).broadcast_to([GPB, N, 6])
                i6n = iota6f[0:GPB, :].rearrange("g (o c) -> g o c", o=1).broadcast_to([GPB, N, 6])
                nc.vector.tensor_tensor(xoh3, xcn_r, i6n, op=AL.is_equal)
                poh = np_.tile([GPB, N * C6], F32)
                pvn_r = pvn[:].rearrange("g (n o) -> g n o", o=1).broadcast_to([GPB, N, 6])
                nc.vector.tensor_tensor(poh[:].rearrange("g (n c) -> g n c", c=6),
                                        pvn_r, i6n, op=AL.is_equal)
                seln_r = seln[:].rearrange("g (n o) -> g n o", o=1).broadcast_to([GPB, N, 6])
                nc.vector.copy_predicated(xoh3, seln_r,
                                          poh[:].rearrange("g (n c) -> g n c", c=6))
                nc.sync.dma_start(out=xnew[:].rearrange("g n c -> g (n c)"), in_=xoh[:])

            # ===================== edges =====================
            with (
                tc.tile_pool(name="persist", bufs=1) as pp,
                tc.tile_pool(name="stream", bufs=2) as sp,
                tc.tile_pool(name="vec", bufs=2) as vp,
                tc.tile_pool(name="small", bufs=2) as mp,
                tc.tile_pool(name="psum", bufs=1, space="PSUM") as qp,
            ):
                ecl_t = [pp.tile([128, 4 * N], I32, name=f"ecl{g}", tag=f"ecl{g}")
                         for g in range(GPB)]
                # key tiles are reused as U (upper-tri value matrix) in phase D
                key_t = [pp.tile([128, 4 * N], F32, name=f"key{g}", tag=f"key{g}")
                         for g in range(GPB)]
                sel_t = [pp.tile([128, 4 * N], I8, name=f"sel{g}", tag=f"sel{g}")
                         for g in range(GPB)]
                cand = pp.tile([128, GPB, 256], F32)
                cntp = pp.tile([128, GPB], F32)

                # ---- phase A: keys + candidates + counts ----
                for g in range(GPB):
                    nc.sync.dma_start(
                        out=ecl_t[g][:].rearrange("p (t j) -> p t j", t=4),
                        in_=ecl[g].rearrange("(t p) j -> p t j", p=128))
                    gtile = sp.tile([128, 4 * N], F32, name="ged", tag="ged")
                    nc.sync.dma_start(
                        out=gtile[:].rearrange("p (t j) -> p t j", t=4),
                        in_=ged[g].rearrange("(t p) j -> p t j", p=128))
                    nc.vector.scalar_tensor_tensor(
                        out=key_t[g][:], in0=ecl_t[g][:], scalar=5.0, in1=gtile[:],
                        op0=AL.is_equal, op1=AL.mult)
                    nc.gpsimd.affine_select(
                        key_t[g][:].rearrange("p (t j) -> p t j", t=4),
                        key_t[g][:].rearrange("p (t j) -> p t j", t=4),
                        pattern=[[-128, 4], [1, N]], compare_op=AL.is_ge, fill=NEG,
                        base=-1, channel_multiplier=-1)
                    scr = sp.tile([128, 4 * N], F32, name="scr", tag="ged")
                    nc.vector.tensor_scalar(
                        out=scr[:], in0=key_t[g][:], scalar1=0.0, scalar2=None,
                        op0=AL.is_gt)
                    nc.vector.tensor_reduce(cntp[:, g:g + 1], scr[:], axis=AX.X,
                                            op=AL.add)
                    for u in range(32):
                        nc.vector.max(cand[:, g, 8 * u:8 * u + 8],
                                      key_t[g][:, 64 * u:64 * (u + 1)])

                cnt_ps = qp.tile([128, GPB], F32, bufs=1)
                nc.tensor.matmul(cnt_ps[:], ones[:], cntp[:])
                ktile = mp.tile([128, GPB], F32, tag="ktile")
                nc.vector.tensor_scalar(out=ktile[:], in0=cnt_ps[:],
                                        scalar1=ke_bc[:, 0:1],
                                        scalar2=None, op0=AL.min)

                # ---- phase B: batched float bisection ----
                lo = mp.tile([128, GPB], F32, tag="lo")
                hi = mp.tile([128, GPB], F32, tag="hi")
                mid = mp.tile([128, GPB], F32, tag="mid")
                nc.vector.memset(lo[:], -6.0)
                nc.vector.memset(hi[:], 16.0)
                nc.vector.memset(mid[:], 5.0)
                for it in range(BISECT_ITERS):
                    gtm = vp.tile([128, GPB * 256], F32, tag="gtm")
                    mid_b = mid[:].rearrange("p (g o) -> p g o", o=1).broadcast_to([128, GPB, 256])
                    nc.vector.tensor_tensor(gtm[:].rearrange("p (g c) -> p g c", g=GPB),
                                            cand[:], mid_b, op=AL.is_gt)
                    cc = vp.tile([128, GPB], F32, tag="cc")
                    nc.vector.tensor_reduce(cc[:], gtm[:].rearrange("p (g c) -> p g c", g=GPB),
                                            axis=AX.X, op=AL.add)
                    cps = qp.tile([128, GPB], F32, tag="cps", bufs=2)
                    nc.tensor.matmul(cps[:], ones[:], cc[:])
                    gem = vp.tile([128, GPB], I8, tag="gem")
                    nc.vector.tensor_tensor(gem[:], cps[:], ktile[:], op=AL.is_ge)
                    nc.vector.copy_predicated(lo[:], gem[:], mid[:])
                    ltm = vp.tile([128, GPB], I8, tag="ltm")
                    nc.vector.tensor_scalar(out=ltm[:], in0=gem[:], scalar1=0.5,
                                            scalar2=None, op0=AL.is_lt)
                    nc.vector.copy_predicated(hi[:], ltm[:], mid[:])
                    nc.vector.tensor_add(mid[:], lo[:], hi[:])
                    nc.vector.tensor_scalar_mul(mid[:], mid[:], 0.5)

                # sel = key > lo  (exact: no representable value inside (lo,hi))
                for g in range(GPB):
                    nc.vector.tensor_scalar(out=sel_t[g][:], in0=key_t[g][:],
                                            scalar1=lo[:, g:g + 1], scalar2=None,
                                            op0=AL.is_gt)

                # ---- phase D: edge pred + U (key tiles overwritten as U) ----
                H = N // 2
                for g in range(GPB):
                    for t in range(4):
                        for h in range(2):
                            j0 = H * h
                            lxt = sp.tile([128, H * C6], F32, name="lxt", tag="lxt")
                            nc.sync.dma_start(
                                out=lxt[:],
                                in_=les[g, 128 * t:128 * (t + 1), j0:j0 + H]
                                .rearrange("p j c -> p (j c)"))
                            get_ = sp.tile([128, H * C6], F32, name="get", tag="get")
                            nc.scalar.dma_start(
                                out=get_[:],
                                in_=ge6[g, 128 * t:128 * (t + 1), j0:j0 + H]
                                .rearrange("p j c -> p (j c)"))
                            nc.gpsimd.tensor_add(lxt[:], lxt[:], get_[:])
                            pv = vp.tile([128, H], F32, name="pv", tag="pv")
                            i6eb = (iota6f[:, :].rearrange("p (o c) -> p o c", o=1)
                                    .broadcast_to([128, H, 6]))
                            _argmax6(nc, vp, lxt[:].rearrange("p (j c) -> p j c", c=6),
                                     128, H, pv[:], "e", i6eb)
                            ecf = vp.tile([128, H], F32, name="ecf", tag="ecf")
                            nc.vector.tensor_copy(
                                ecf[:], ecl_t[g][:, N * t + j0:N * t + j0 + H])
                            nc.vector.copy_predicated(
                                ecf[:], sel_t[g][:, N * t + j0:N * t + j0 + H], pv[:])
                            nc.gpsimd.affine_select(
                                key_t[g][:, N * t + j0:N * t + j0 + H], ecf[:],
                                pattern=[[1, H]], compare_op=AL.is_ge, fill=0.0,
                                base=j0 - 128 * t - 1, channel_multiplier=-1)

                # ---- phase E: symmetrize + one-hot + store ----
                for g in range(GPB):
                    for s in range(4):
                        pst = qp.tile([128, N], F32, name="pst", tag="pst", bufs=4)
                        for t in range(4):
                            nc.tensor.transpose(
                                pst[:, 128 * t:128 * (t + 1)],
                                key_t[g][:, N * t + 128 * s:N * t + 128 * (s + 1)],
                                ident[:])
                        symv = sp.tile([128, N], F32, name="symv", tag="symv")
                        nc.vector.tensor_add(symv[:], key_t[g][:, N * s:N * (s + 1)],
                                             pst[:])
                        nc.gpsimd.tensor_add(symv[:], symv[:],
                                             dmask6[:, N * s:N * (s + 1)])
                        for h in range(2):
                            j0 = H * h
                            oh = sp.tile([128, H * C6], F32, name="oh", tag="oh")
                            symr = (symv[:, j0:j0 + H]
                                    .rearrange("p (j o) -> p j o", o=1)
                                    .broadcast_to([128, H, 6]))
                            i6e = (iota6f[:, :].rearrange("p (o c) -> p o c", o=1)
                                   .broadcast_to([128, H, 6]))
                            nc.vector.tensor_tensor(
                                oh[:].rearrange("p (j c) -> p j c", c=6),
                                symr, i6e, op=AL.is_equal)
                            nc.sync.dma_start(
                                out=enew[g, 128 * s:128 * (s + 1), j0:j0 + H]
                                .rearrange("p j c -> p (j c)"),
                                in_=oh[:])
    _split_multi_waits(nc)
    return nc


_NC = None


def make_in_maps(logits_x, logits_e, g_node, g_x, g_edge, g_e, Xclass, Eclass,
                 step_nodes, step_edges):
    B = Xclass.shape[0]
    ncore = 8
    gpc = B // ncore
    in_maps = []
    for c in range(ncore):
        s = slice(c * gpc, (c + 1) * gpc)
        in_maps.append({
            "eclass": np.ascontiguousarray(Eclass[s], dtype=np.int32),
            "gedge": np.ascontiguousarray(g_edge[s], dtype=np.float32),
            "logitse": np.ascontiguousarray(logits_e[s], dtype=np.float32),
            "ge6": np.ascontiguousarray(g_e[s], dtype=np.float32),
            "logitsx": np.ascontiguousarray(logits_x[s], dtype=np.float32),
            "gx6": np.ascontiguousarray(g_x[s], dtype=np.float32),
            "xclass": np.ascontiguousarray(Xclass[s], dtype=np.int32),
            "gnode": np.ascontiguousarray(g_node[s], dtype=np.float32),
            "stepe": np.full((128, 1), float(step_edges), dtype=np.float32),
            "stepn": np.full((128, 1), float(step_nodes), dtype=np.float32),
        })
    return in_maps


def kernel(logits_x, logits_e, g_node, g_x, g_edge, g_e, Xclass, Eclass,
           step_nodes, step_edges):
    global _NC
    ncore = 8
    assert Xclass.shape[0] // ncore == GPB
    if _NC is None:
        _NC = build_kernel()
    in_maps = make_in_maps(logits_x, logits_e, g_node, g_x, g_edge, g_e,
                           Xclass, Eclass, step_nodes, step_edges)
    res = run_bass_kernel_spmd(_NC, in_maps, core_ids=list(range(ncore)))
    Xnew = np.concatenate([res.results[c]["xnew"] for c in range(ncore)], axis=0)
    Enew = np.concatenate([res.results[c]["enew"] for c in range(ncore)], axis=0)
    return Xnew, Enew


# revision 16
# speedup vs baseline: 1.0302x; 1.0302x over previous
"""AbsorbingGraphKernel for trn2: data-parallel over batch (4 graphs/core x 8 cores).

Per graph:
  nodes: top-k_n absorbed gumbel -> unmask -> argmax(logits_x+g_x) -> one-hot
  edges: top-k_e absorbed gumbel on triu -> unmask -> argmax(logits_e+g_e)
         -> symmetrize -> one-hot with zeroed diagonal
Exact top-k threshold found by float bisection over per-chunk top-8 candidates.
"""
import numpy as np

import concourse.bass as bass
import concourse.mybir as mybir
import concourse.tile as tile
from concourse.bass_utils import run_bass_kernel_spmd

# this walrus build cannot codegen the EventSemaphore butterfly barrier nor
# a Drain carrying sem waits; NRT drains DMA at execution end, so skip both.
tile.TileContext._drain_and_barrier = lambda self, tick_clock, wait_clock: None

F32 = mybir.dt.float32
I32 = mybir.dt.int32
I8 = mybir.dt.int8
AL = mybir.AluOpType
AX = mybir.AxisListType

GPB = 4          # graphs per core
N = 512
C6 = 6
NEG = -1.0e9
BISECT_ITERS = 30


def _argmax6(nc, vpool, src3, P, width, out_pv, tagp, iota6bc):
    """argmax over last dim of src3 [P, width, 6] -> out_pv [P,width] f32, first-index ties."""
    ls = [src3[:, :, c] for c in range(6)]
    mk = lambda i: vpool.tile([P, width], F32, name=f"amx{i}", tag=f"amx{i}{tagp}")
    m01, m23, m45, m0123 = mk(0), mk(1), mk(2), mk(3)
    i01, i23, i45, b = mk(4), mk(5), mk(6), mk(9)
    s2 = vpool.tile([P, width], I8, name="amx7", tag=f"amx7{tagp}")
    s3 = vpool.tile([P, width], I8, name="amx8", tag=f"amx8{tagp}")
    nc.vector.tensor_max(m01[:], ls[0], ls[1])
    nc.vector.tensor_max(m23[:], ls[2], ls[3])
    nc.vector.tensor_max(m45[:], ls[4], ls[5])
    nc.vector.tensor_tensor(i01[:], ls[0], ls[1], op=AL.is_lt)
    nc.vector.tensor_tensor(i23[:], ls[2], ls[3], op=AL.is_lt)
    nc.vector.tensor_tensor(i45[:], ls[4], ls[5], op=AL.is_lt)
    nc.vector.tensor_tensor(s2[:], m01[:], m23[:], op=AL.is_lt)
    nc.vector.tensor_max(m0123[:], m01[:], m23[:])
    nc.vector.tensor_tensor(s3[:], m0123[:], m45[:], op=AL.is_lt)
    nc.vector.tensor_scalar_add(b[:], i23[:], 2.0)
    nc.vector.tensor_copy(out_pv, i01[:])
    nc.vector.copy_predicated(out_pv, s2[:], b[:])
    nc.vector.tensor_scalar_add(b[:], i45[:], 4.0)
    nc.vector.copy_predicated(out_pv, s3[:], b[:])


def _split_multi_waits(nc):
    """This walrus codegen accepts at most ONE sem wait per instruction.
    Tile emits joins with several; peel extras onto EventSemaphore carriers
    inserted immediately before, on the same engine (same program point, so
    semantics are identical)."""
    import copy
    tmpl = None
    for b in nc.m.functions[0].blocks:
        for i in b.instructions:
            if str(i.opcode) == "EventSemaphore" and i.sync_info is not None:
                tmpl = i
                break
        if tmpl is not None:
            break
    assert tmpl is not None, "no EventSemaphore template found"
    cnt = 0
    for f in nc.m.functions:
        for b in f.blocks:
            out = []
            changed = False
            for i in b.instructions:
                si = i.sync_info
                if si is not None and len(si.on_wait) > 1:
                    waits = [copy.deepcopy(w) for w in si.on_wait]
                    for w in waits[:-1]:
                        cnt += 1
                        c = copy.deepcopy(tmpl)
                        c.name = f"waitc_{cnt}"
                        c.engine = i.engine
                        csi = c.sync_info
                        csi.on_wait = [w]
                        csi.on_update = []
                        c.sync_info = csi
                        out.append(c)
                    si.on_wait = [waits[-1]]
                    i.sync_info = si
                    changed = True
                out.append(i)
            if changed:
                b.instructions = out


def build_kernel():
    nc = bass.Bass()
    ecl = nc.declare_dram_parameter("eclass", [GPB, N, N], I32, isOutput=False)
    ged = nc.declare_dram_parameter("gedge", [GPB, N, N], F32, isOutput=False)
    les = nc.declare_dram_parameter("logitse", [GPB, N, N, C6], F32, isOutput=False)
    ge6 = nc.declare_dram_parameter("ge6", [GPB, N, N, C6], F32, isOutput=False)
    lxs = nc.declare_dram_parameter("logitsx", [GPB, N, C6], F32, isOutput=False)
    gx6 = nc.declare_dram_parameter("gx6", [GPB, N, C6], F32, isOutput=False)
    xcl = nc.declare_dram_parameter("xclass", [GPB, N], I32, isOutput=False)
    gnd = nc.declare_dram_parameter("gnode", [GPB, N], F32, isOutput=False)
    ste = nc.declare_dram_parameter("stepe", [128, 1], F32, isOutput=False)
    stn = nc.declare_dram_parameter("stepn", [128, 1], F32, isOutput=False)
    enew = nc.declare_dram_parameter("enew", [GPB, N, N, C6], F32, isOutput=True)
    xnew = nc.declare_dram_parameter("xnew", [GPB, N, C6], F32, isOutput=True)

    with tile.TileContext(nc) as tc:
        with tc.tile_pool(name="const", bufs=1) as cp:
            ident = cp.tile([128, 128], F32)
            nc.vector.memset(ident[:], 1.0)
            nc.gpsimd.affine_select(ident[:], ident[:], pattern=[[1, 128]],
                                    compare_op=AL.is_equal, fill=0.0,
                                    base=0, channel_multiplier=-1)
            ones = cp.tile([128, 128], F32)
            nc.vector.memset(ones[:], 1.0)
            iota6i = cp.tile([128, 6], I32)
            nc.gpsimd.iota(iota6i[:], pattern=[[1, 6]], base=0, channel_multiplier=0)
            iota6f = cp.tile([128, 6], F32)
            nc.vector.tensor_copy(iota6f[:], iota6i[:])
            iota8i = cp.tile([128, 8], I32)
            nc.gpsimd.iota(iota8i[:], pattern=[[1, 8]], base=0, channel_multiplier=0)
            iota8f = cp.tile([128, 8], F32)
            nc.vector.tensor_copy(iota8f[:], iota8i[:])
            negm = cp.tile([128, 4 * N], F32)
            nc.vector.memset(negm[:], 0.0)
            nc.gpsimd.affine_select(
                negm[:].rearrange("p (t j) -> p t j", t=4),
                negm[:].rearrange("p (t j) -> p t j", t=4),
                pattern=[[-128, 4], [1, N]], compare_op=AL.is_ge, fill=NEG,
                base=-1, channel_multiplier=-1)
            dmask6 = cp.tile([128, 4 * N], F32)
            nc.vector.memset(dmask6[:], 6.0)
            nc.gpsimd.affine_select(
                dmask6[:].rearrange("p (s j) -> p s j", s=4),
                dmask6[:].rearrange("p (s j) -> p s j", s=4),
                pattern=[[-128, 4], [1, N]], compare_op=AL.is_equal, fill=0.0,
                base=0, channel_multiplier=-1)
            ke_bc = cp.tile([128, 1], F32)
            nc.sync.dma_start(out=ke_bc[:], in_=ste[:])
            kn_bc = cp.tile([128, 1], F32)
            nc.sync.dma_start(out=kn_bc[:], in_=stn[:])

            # ===================== nodes (separate scope) =====================
            with tc.tile_pool(name="nodes", bufs=1) as np_:
                lxn = np_.tile([GPB, N * C6], F32)
                nc.sync.dma_start(out=lxn[:], in_=lxs[:].rearrange("g n c -> g (n c)"))
                gxn = np_.tile([GPB, N * C6], F32)
                nc.sync.dma_start(out=gxn[:], in_=gx6[:].rearrange("g n c -> g (n c)"))
                xcn = np_.tile([GPB, N], I32)
                nc.sync.dma_start(out=xcn[:], in_=xcl[:])
                gnn = np_.tile([GPB, N], F32)
                nc.sync.dma_start(out=gnn[:], in_=gnd[:])

                keyn = np_.tile([GPB, N], F32)
                cntn = np_.tile([GPB, 1], F32)
                nscr = np_.tile([GPB, N], F32)
                nc.vector.tensor_scalar(out=nscr[:], in0=xcn[:], scalar1=5.0,
                                        scalar2=None, op0=AL.is_equal)
                nc.vector.tensor_reduce(cntn[:], nscr[:], axis=AX.X, op=AL.add)
                nc.vector.tensor_mul(keyn[:], nscr[:], gnn[:])
                top8 = np_.tile([GPB, 8], F32)
                nc.vector.max(top8[:], keyn[:])
                kn = np_.tile([GPB, 1], F32)
                nc.vector.tensor_scalar(out=kn[:], in0=cntn[:], scalar1=kn_bc[0:GPB, :],
                                        scalar2=None, op0=AL.min)
                knm1 = np_.tile([GPB, 1], F32)
                nc.vector.tensor_scalar(out=knm1[:], in0=kn[:], scalar1=1.0,
                                        scalar2=None, op0=AL.subtract)
                eqk = np_.tile([GPB, 8], F32)
                nc.vector.tensor_scalar(out=eqk[:], in0=iota8f[0:GPB, :], scalar1=knm1[:],
                                        scalar2=None, op0=AL.is_equal)
                thn8 = np_.tile([GPB, 8], F32)
                nc.vector.tensor_mul(thn8[:], top8[:], eqk[:])
                thn = np_.tile([GPB, 1], F32)
                nc.vector.tensor_reduce(thn[:], thn8[:], axis=AX.X, op=AL.add)
                seln = np_.tile([GPB, N], I8)
                nc.vector.tensor_scalar(out=seln[:], in0=keyn[:], scalar1=thn[:],
                                        scalar2=None, op0=AL.is_ge)

                nc.vector.tensor_add(lxn[:], lxn[:], gxn[:])
                pvn = np_.tile([GPB, N], F32)
                i6nb = (iota6f[0:GPB, :].rearrange("g (o c) -> g o c", o=1)
                        .broadcast_to([GPB, N, 6]))
                _argmax6(nc, np_, lxn[:].rearrange("g (n c) -> g n c", c=6),
                         GPB, N, pvn[:], "n", i6nb)
                xoh = np_.tile([GPB, N * C6], F32)
                xoh3 = xoh[:].rearrange("g (n c) -> g n c", c=6)
                xcn_r = xcn[:].rearrange("g (n o) -> g n o", o=1).broadcast_to([GPB, N, 6])
                i6n = iota6f[0:GPB, :].rearrange("g (o c) -> g o c", o=1).broadcast_to([GPB, N, 6])
                nc.vector.tensor_tensor(xoh3, xcn_r, i6n, op=AL.is_equal)
                poh = np_.tile([GPB, N * C6], F32)
                pvn_r = pvn[:].rearrange("g (n o) -> g n o", o=1).broadcast_to([GPB, N, 6])
                nc.vector.tensor_tensor(poh[:].rearrange("g (n c) -> g n c", c=6),
                                        pvn_r, i6n, op=AL.is_equal)
                seln_r = seln[:].rearrange("g (n o) -> g n o", o=1).broadcast_to([GPB, N, 6])
                nc.vector.copy_predicated(xoh3, seln_r,
                                          poh[:].rearrange("g (n c) -> g n c", c=6))
                nc.sync.dma_start(out=xnew[:].rearrange("g n c -> g (n c)"), in_=xoh[:])

            # ===================== edges =====================
            with (
                tc.tile_pool(name="persist", bufs=1) as pp,
                tc.tile_pool(name="stream", bufs=2) as sp,
                tc.tile_pool(name="vec", bufs=2) as vp,
                tc.tile_pool(name="small", bufs=2) as mp,
                tc.tile_pool(name="psum", bufs=1, space="PSUM") as qp,
            ):
                ecl_t = [pp.tile([128, 4 * N], I32, name=f"ecl{g}", tag=f"ecl{g}")
                         for g in range(GPB)]
                # key tiles are reused as U (upper-tri value matrix) in phase D
                key_t = [pp.tile([128, 4 * N], F32, name=f"key{g}", tag=f"key{g}")
                         for g in range(GPB)]
                sel_t = [pp.tile([128, 4 * N], I8, name=f"sel{g}", tag=f"sel{g}")
                         for g in range(GPB)]
                cand = pp.tile([128, GPB, 256], F32)
                cntp = pp.tile([128, GPB], F32)

                # ---- phase A: keys + candidates + counts ----
                for g in range(GPB):
                    nc.sync.dma_start(
                        out=ecl_t[g][:].rearrange("p (t j) -> p t j", t=4),
                        in_=ecl[g].rearrange("(t p) j -> p t j", p=128))
                    gtile = sp.tile([128, 4 * N], F32, name="ged", tag="ged")
                    nc.sync.dma_start(
                        out=gtile[:].rearrange("p (t j) -> p t j", t=4),
                        in_=ged[g].rearrange("(t p) j -> p t j", p=128))
                    nc.vector.scalar_tensor_tensor(
                        out=key_t[g][:], in0=ecl_t[g][:], scalar=5.0, in1=gtile[:],
                        op0=AL.is_equal, op1=AL.mult)
                    nc.vector.tensor_add(key_t[g][:], key_t[g][:], negm[:])
                    scr = sp.tile([128, 4 * N], F32, name="scr", tag="ged")
                    nc.vector.tensor_scalar(
                        out=scr[:], in0=key_t[g][:], scalar1=0.0, scalar2=None,
                        op0=AL.is_gt)
                    nc.vector.tensor_reduce(cntp[:, g:g + 1], scr[:], axis=AX.X,
                                            op=AL.add)
                    for u in range(32):
                        nc.vector.max(cand[:, g, 8 * u:8 * u + 8],
                                      key_t[g][:, 64 * u:64 * (u + 1)])

                cnt_ps = qp.tile([128, GPB], F32, bufs=1)
                nc.tensor.matmul(cnt_ps[:], ones[:], cntp[:])
                ktile = mp.tile([128, GPB], F32, tag="ktile")
                nc.vector.tensor_scalar(out=ktile[:], in0=cnt_ps[:],
                                        scalar1=ke_bc[:, 0:1],
                                        scalar2=None, op0=AL.min)

                # ---- phase B: batched float bisection ----
                lo = mp.tile([128, GPB], F32, tag="lo")
                hi = mp.tile([128, GPB], F32, tag="hi")
                mid = mp.tile([128, GPB], F32, tag="mid")
                nc.vector.memset(lo[:], -6.0)
                nc.vector.memset(hi[:], 16.0)
                nc.vector.memset(mid[:], 5.0)
                for it in range(BISECT_ITERS):
                    gtm = vp.tile([128, GPB * 256], F32, tag="gtm")
                    mid_b = mid[:].rearrange("p (g o) -> p g o", o=1).broadcast_to([128, GPB, 256])
                    nc.vector.tensor_tensor(gtm[:].rearrange("p (g c) -> p g c", g=GPB),
                                            cand[:], mid_b, op=AL.is_gt)
                    cc = vp.tile([128, GPB], F32, tag="cc")
                    nc.vector.tensor_reduce(cc[:], gtm[:].rearrange("p (g c) -> p g c", g=GPB),
                                            axis=AX.X, op=AL.add)
                    cps = qp.tile([128, GPB], F32, tag="cps", bufs=2)
                    nc.tensor.matmul(cps[:], ones[:], cc[:])
                    gem = vp.tile([128, GPB], I8, tag="gem")
                    nc.vector.tensor_tensor(gem[:], cps[:], ktile[:], op=AL.is_ge)
                    nc.vector.copy_predicated(lo[:], gem[:], mid[:])
                    ltm = vp.tile([128, GPB], I8, tag="ltm")
                    nc.vector.tensor_scalar(out=ltm[:], in0=gem[:], scalar1=0.5,
                                            scalar2=None, op0=AL.is_lt)
                    nc.vector.copy_predicated(hi[:], ltm[:], mid[:])
                    nc.vector.tensor_add(mid[:], lo[:], hi[:])
                    nc.vector.tensor_scalar_mul(mid[:], mid[:], 0.5)

                # sel = key > lo  (exact: no representable value inside (lo,hi))
                for g in range(GPB):
                    nc.vector.tensor_scalar(out=sel_t[g][:], in0=key_t[g][:],
                                            scalar1=lo[:, g:g + 1], scalar2=None,
                                            op0=AL.is_gt)

                # ---- phase D: edge pred + U (key tiles overwritten as U) ----
                H = N // 2
                for g in range(GPB):
                    for t in range(4):
                        for h in range(2):
                            j0 = H * h
                            lxt = sp.tile([128, H * C6], F32, name="lxt", tag="lxt")
                            nc.sync.dma_start(
                                out=lxt[:],
                                in_=les[g, 128 * t:128 * (t + 1), j0:j0 + H]
                                .rearrange("p j c -> p (j c)"))
                            get_ = sp.tile([128, H * C6], F32, name="get", tag="get")
                            nc.scalar.dma_start(
                                out=get_[:],
                                in_=ge6[g, 128 * t:128 * (t + 1), j0:j0 + H]
                                .rearrange("p j c -> p (j c)"))
                            nc.gpsimd.tensor_add(lxt[:], lxt[:], get_[:])
                            pv = vp.tile([128, H], F32, name="pv", tag="pv")
                            i6eb = (iota6f[:, :].rearrange("p (o c) -> p o c", o=1)
                                    .broadcast_to([128, H, 6]))
                            _argmax6(nc, vp, lxt[:].rearrange("p (j c) -> p j c", c=6),
                                     128, H, pv[:], "e", i6eb)
                            ecf = vp.tile([128, H], F32, name="ecf", tag="ecf")
                            nc.vector.tensor_copy(
                                ecf[:], ecl_t[g][:, N * t + j0:N * t + j0 + H])
                            nc.vector.copy_predicated(
                                ecf[:], sel_t[g][:, N * t + j0:N * t + j0 + H], pv[:])
                            nc.gpsimd.affine_select(
                                key_t[g][:, N * t + j0:N * t + j0 + H], ecf[:],
                                pattern=[[1, H]], compare_op=AL.is_ge, fill=0.0,
                                base=j0 - 128 * t - 1, channel_multiplier=-1)

                # ---- phase E: symmetrize + one-hot + store ----
                for g in range(GPB):
                    for s in range(4):
                        pst = qp.tile([128, N], F32, name="pst", tag="pst", bufs=4)
                        for t in range(4):
                            nc.tensor.transpose(
                                pst[:, 128 * t:128 * (t + 1)],
                                key_t[g][:, N * t + 128 * s:N * t + 128 * (s + 1)],
                                ident[:])
                        symv = sp.tile([128, N], F32, name="symv", tag="symv")
                        nc.vector.tensor_add(symv[:], key_t[g][:, N * s:N * (s + 1)],
                                             pst[:])
                        nc.gpsimd.tensor_add(symv[:], symv[:],
                                             dmask6[:, N * s:N * (s + 1)])
                        for h in range(2):
                            j0 = H * h
                            oh = sp.tile([128, H * C6], F32, name="oh", tag="oh")
                            symr = (symv[:, j0:j0 + H]
                                    .rearrange("p (j o) -> p j o", o=1)
                                    .broadcast_to([128, H, 6]))
                            i6e = (iota6f[:, :].rearrange("p (o c) -> p o c", o=1)
                                   .broadcast_to([128, H, 6]))
                            nc.vector.tensor_tensor(
                                oh[:].rearrange("p (j c) -> p j c", c=6),
                                symr, i6e, op=AL.is_equal)
                            nc.sync.dma_start(
                                out=enew[g, 128 * s:128 * (s + 1), j0:j0 + H]
                                .rearrange("p j c -> p (j c)"),
                                in_=oh[:])
    _split_multi_waits(nc)
    return nc


_NC = None


def make_in_maps(logits_x, logits_e, g_node, g_x, g_edge, g_e, Xclass, Eclass,
                 step_nodes, step_edges):
    B = Xclass.shape[0]
    ncore = 8
    gpc = B // ncore
    in_maps = []
    for c in range(ncore):
        s = slice(c * gpc, (c + 1) * gpc)
        in_maps.append({
            "eclass": np.ascontiguousarray(Eclass[s], dtype=np.int32),
            "gedge": np.ascontiguousarray(g_edge[s], dtype=np.float32),
            "logitse": np.ascontiguousarray(logits_e[s], dtype=np.float32),
            "ge6": np.ascontiguousarray(g_e[s], dtype=np.float32),
            "logitsx": np.ascontiguousarray(logits_x[s], dtype=np.float32),
            "gx6": np.ascontiguousarray(g_x[s], dtype=np.float32),
            "xclass": np.ascontiguousarray(Xclass[s], dtype=np.int32),
            "gnode": np.ascontiguousarray(g_node[s], dtype=np.float32),
            "stepe": np.full((128, 1), float(step_edges), dtype=np.float32),
            "stepn": np.full((128, 1), float(step_nodes), dtype=np.float32),
        })
    return in_maps


def kernel(logits_x, logits_e, g_node, g_x, g_edge, g_e, Xclass, Eclass,
           step_nodes, step_edges):
    global _NC
    ncore = 8
    assert Xclass.shape[0] // ncore == GPB
    if _NC is None:
        _NC = build_kernel()
    in_maps = make_in_maps(logits_x, logits_e, g_node, g_x, g_edge, g_e,
                           Xclass, Eclass, step_nodes, step_edges)
    res = run_bass_kernel_spmd(_NC, in_maps, core_ids=list(range(ncore)))
    Xnew = np.concatenate([res.results[c]["xnew"] for c in range(ncore)], axis=0)
    Enew = np.concatenate([res.results[c]["enew"] for c in range(ncore)], axis=0)
    return Xnew, Enew


# revision 17
# speedup vs baseline: 1.1358x; 1.1025x over previous
"""AbsorbingGraphKernel for trn2: data-parallel over batch (4 graphs/core x 8 cores).

Per graph:
  nodes: top-k_n absorbed gumbel -> unmask -> argmax(logits_x+g_x) -> one-hot
  edges: top-k_e absorbed gumbel on triu -> unmask -> argmax(logits_e+g_e)
         -> symmetrize -> one-hot with zeroed diagonal
Exact top-k threshold found by float bisection over per-chunk top-8 candidates.
"""
import numpy as np

import concourse.bass as bass
import concourse.mybir as mybir
import concourse.tile as tile
from concourse.bass_utils import run_bass_kernel_spmd

# this walrus build cannot codegen the EventSemaphore butterfly barrier nor
# a Drain carrying sem waits; NRT drains DMA at execution end, so skip both.
tile.TileContext._drain_and_barrier = lambda self, tick_clock, wait_clock: None

F32 = mybir.dt.float32
I32 = mybir.dt.int32
I8 = mybir.dt.int8
AL = mybir.AluOpType
AX = mybir.AxisListType

GPB = 4          # graphs per core
N = 512
C6 = 6
NEG = -1.0e9
BISECT_ITERS = 30


def _argmax6(nc, vpool, src3, P, width, out_pv, tagp, iota6bc):
    """argmax over last dim of src3 [P, width, 6] -> out_pv [P,width] f32, first-index ties."""
    ls = [src3[:, :, c] for c in range(6)]
    mk = lambda i: vpool.tile([P, width], F32, name=f"amx{i}", tag=f"amx{i}{tagp}")
    m01, m23, m45, m0123 = mk(0), mk(1), mk(2), mk(3)
    i01, i23, i45, b = mk(4), mk(5), mk(6), mk(9)
    s2 = vpool.tile([P, width], I8, name="amx7", tag=f"amx7{tagp}")
    s3 = vpool.tile([P, width], I8, name="amx8", tag=f"amx8{tagp}")
    nc.vector.tensor_max(m01[:], ls[0], ls[1])
    nc.vector.tensor_max(m23[:], ls[2], ls[3])
    nc.vector.tensor_max(m45[:], ls[4], ls[5])
    nc.vector.tensor_tensor(i01[:], ls[0], ls[1], op=AL.is_lt)
    nc.vector.tensor_tensor(i23[:], ls[2], ls[3], op=AL.is_lt)
    nc.vector.tensor_tensor(i45[:], ls[4], ls[5], op=AL.is_lt)
    nc.vector.tensor_tensor(s2[:], m01[:], m23[:], op=AL.is_lt)
    nc.vector.tensor_max(m0123[:], m01[:], m23[:])
    nc.vector.tensor_tensor(s3[:], m0123[:], m45[:], op=AL.is_lt)
    nc.vector.tensor_scalar_add(b[:], i23[:], 2.0)
    nc.vector.tensor_copy(out_pv, i01[:])
    nc.vector.copy_predicated(out_pv, s2[:], b[:])
    nc.vector.tensor_scalar_add(b[:], i45[:], 4.0)
    nc.vector.copy_predicated(out_pv, s3[:], b[:])


def _split_multi_waits(nc):
    """This walrus codegen accepts at most ONE sem wait per instruction.
    Tile emits joins with several; peel extras onto EventSemaphore carriers
    inserted immediately before, on the same engine (same program point, so
    semantics are identical)."""
    import copy
    tmpl = None
    for b in nc.m.functions[0].blocks:
        for i in b.instructions:
            if str(i.opcode) == "EventSemaphore" and i.sync_info is not None:
                tmpl = i
                break
        if tmpl is not None:
            break
    assert tmpl is not None, "no EventSemaphore template found"
    cnt = 0
    for f in nc.m.functions:
        for b in f.blocks:
            out = []
            changed = False
            for i in b.instructions:
                si = i.sync_info
                if si is not None and len(si.on_wait) > 1:
                    waits = [copy.deepcopy(w) for w in si.on_wait]
                    for w in waits[:-1]:
                        cnt += 1
                        c = copy.deepcopy(tmpl)
                        c.name = f"waitc_{cnt}"
                        c.engine = i.engine
                        csi = c.sync_info
                        csi.on_wait = [w]
                        csi.on_update = []
                        c.sync_info = csi
                        out.append(c)
                    si.on_wait = [waits[-1]]
                    i.sync_info = si
                    changed = True
                out.append(i)
            if changed:
                b.instructions = out


def build_kernel():
    nc = bass.Bass()
    ecl = nc.declare_dram_parameter("eclass", [GPB, N, N], I32, isOutput=False)
    ged = nc.declare_dram_parameter("gedge", [GPB, N, N], F32, isOutput=False)
    les = nc.declare_dram_parameter("logitse", [GPB, N, N, C6], F32, isOutput=False)
    ge6 = nc.declare_dram_parameter("ge6", [GPB, N, N, C6], F32, isOutput=False)
    lxs = nc.declare_dram_parameter("logitsx", [GPB, N, C6], F32, isOutput=False)
    gx6 = nc.declare_dram_parameter("gx6", [GPB, N, C6], F32, isOutput=False)
    xcl = nc.declare_dram_parameter("xclass", [GPB, N], I32, isOutput=False)
    gnd = nc.declare_dram_parameter("gnode", [GPB, N], F32, isOutput=False)
    ste = nc.declare_dram_parameter("stepe", [128, 1], F32, isOutput=False)
    stn = nc.declare_dram_parameter("stepn", [128, 1], F32, isOutput=False)
    enew = nc.declare_dram_parameter("enew", [GPB, N, N, C6], F32, isOutput=True)
    xnew = nc.declare_dram_parameter("xnew", [GPB, N, C6], F32, isOutput=True)

    with tile.TileContext(nc) as tc:
        with tc.tile_pool(name="const", bufs=1) as cp:
            ident = cp.tile([128, 128], F32)
            nc.vector.memset(ident[:], 1.0)
            nc.gpsimd.affine_select(ident[:], ident[:], pattern=[[1, 128]],
                                    compare_op=AL.is_equal, fill=0.0,
                                    base=0, channel_multiplier=-1)
            ones = cp.tile([128, 128], F32)
            nc.vector.memset(ones[:], 1.0)
            iota6i = cp.tile([128, 6], I32)
            nc.gpsimd.iota(iota6i[:], pattern=[[1, 6]], base=0, channel_multiplier=0)
            iota6f = cp.tile([128, 6], F32)
            nc.vector.tensor_copy(iota6f[:], iota6i[:])
            iota8i = cp.tile([128, 8], I32)
            nc.gpsimd.iota(iota8i[:], pattern=[[1, 8]], base=0, channel_multiplier=0)
            iota8f = cp.tile([128, 8], F32)
            nc.vector.tensor_copy(iota8f[:], iota8i[:])
            negm = cp.tile([128, 4 * N], F32)
            nc.vector.memset(negm[:], 0.0)
            nc.gpsimd.affine_select(
                negm[:].rearrange("p (t j) -> p t j", t=4),
                negm[:].rearrange("p (t j) -> p t j", t=4),
                pattern=[[-128, 4], [1, N]], compare_op=AL.is_ge, fill=NEG,
                base=-1, channel_multiplier=-1)
            tmask = cp.tile([128, 4 * N], F32)
            nc.vector.memset(tmask[:], 1.0)
            nc.gpsimd.affine_select(
                tmask[:].rearrange("p (t j) -> p t j", t=4),
                tmask[:].rearrange("p (t j) -> p t j", t=4),
                pattern=[[-128, 4], [1, N]], compare_op=AL.is_ge, fill=0.0,
                base=-1, channel_multiplier=-1)
            dmask6 = cp.tile([128, 4 * N], F32)
            nc.vector.memset(dmask6[:], 6.0)
            nc.gpsimd.affine_select(
                dmask6[:].rearrange("p (s j) -> p s j", s=4),
                dmask6[:].rearrange("p (s j) -> p s j", s=4),
                pattern=[[-128, 4], [1, N]], compare_op=AL.is_equal, fill=0.0,
                base=0, channel_multiplier=-1)
            ke_bc = cp.tile([128, 1], F32)
            nc.sync.dma_start(out=ke_bc[:], in_=ste[:])
            kn_bc = cp.tile([128, 1], F32)
            nc.sync.dma_start(out=kn_bc[:], in_=stn[:])

            # ===================== nodes (separate scope) =====================
            with tc.tile_pool(name="nodes", bufs=1) as np_:
                lxn = np_.tile([GPB, N * C6], F32)
                nc.sync.dma_start(out=lxn[:], in_=lxs[:].rearrange("g n c -> g (n c)"))
                gxn = np_.tile([GPB, N * C6], F32)
                nc.sync.dma_start(out=gxn[:], in_=gx6[:].rearrange("g n c -> g (n c)"))
                xcn = np_.tile([GPB, N], I32)
                nc.sync.dma_start(out=xcn[:], in_=xcl[:])
                gnn = np_.tile([GPB, N], F32)
                nc.sync.dma_start(out=gnn[:], in_=gnd[:])

                keyn = np_.tile([GPB, N], F32)
                cntn = np_.tile([GPB, 1], F32)
                nscr = np_.tile([GPB, N], F32)
                nc.vector.tensor_scalar(out=nscr[:], in0=xcn[:], scalar1=5.0,
                                        scalar2=None, op0=AL.is_equal)
                nc.vector.tensor_reduce(cntn[:], nscr[:], axis=AX.X, op=AL.add)
                nc.vector.tensor_mul(keyn[:], nscr[:], gnn[:])
                top8 = np_.tile([GPB, 8], F32)
                nc.vector.max(top8[:], keyn[:])
                kn = np_.tile([GPB, 1], F32)
                nc.vector.tensor_scalar(out=kn[:], in0=cntn[:], scalar1=kn_bc[0:GPB, :],
                                        scalar2=None, op0=AL.min)
                knm1 = np_.tile([GPB, 1], F32)
                nc.vector.tensor_scalar(out=knm1[:], in0=kn[:], scalar1=1.0,
                                        scalar2=None, op0=AL.subtract)
                eqk = np_.tile([GPB, 8], F32)
                nc.vector.tensor_scalar(out=eqk[:], in0=iota8f[0:GPB, :], scalar1=knm1[:],
                                        scalar2=None, op0=AL.is_equal)
                thn8 = np_.tile([GPB, 8], F32)
                nc.vector.tensor_mul(thn8[:], top8[:], eqk[:])
                thn = np_.tile([GPB, 1], F32)
                nc.vector.tensor_reduce(thn[:], thn8[:], axis=AX.X, op=AL.add)
                seln = np_.tile([GPB, N], I8)
                nc.vector.tensor_scalar(out=seln[:], in0=keyn[:], scalar1=thn[:],
                                        scalar2=None, op0=AL.is_ge)

                nc.vector.tensor_add(lxn[:], lxn[:], gxn[:])
                pvn = np_.tile([GPB, N], F32)
                i6nb = (iota6f[0:GPB, :].rearrange("g (o c) -> g o c", o=1)
                        .broadcast_to([GPB, N, 6]))
                _argmax6(nc, np_, lxn[:].rearrange("g (n c) -> g n c", c=6),
                         GPB, N, pvn[:], "n", i6nb)
                xoh = np_.tile([GPB, N * C6], F32)
                xoh3 = xoh[:].rearrange("g (n c) -> g n c", c=6)
                xcn_r = xcn[:].rearrange("g (n o) -> g n o", o=1).broadcast_to([GPB, N, 6])
                i6n = iota6f[0:GPB, :].rearrange("g (o c) -> g o c", o=1).broadcast_to([GPB, N, 6])
                nc.vector.tensor_tensor(xoh3, xcn_r, i6n, op=AL.is_equal)
                poh = np_.tile([GPB, N * C6], F32)
                pvn_r = pvn[:].rearrange("g (n o) -> g n o", o=1).broadcast_to([GPB, N, 6])
                nc.vector.tensor_tensor(poh[:].rearrange("g (n c) -> g n c", c=6),
                                        pvn_r, i6n, op=AL.is_equal)
                seln_r = seln[:].rearrange("g (n o) -> g n o", o=1).broadcast_to([GPB, N, 6])
                nc.vector.copy_predicated(xoh3, seln_r,
                                          poh[:].rearrange("g (n c) -> g n c", c=6))
                nc.sync.dma_start(out=xnew[:].rearrange("g n c -> g (n c)"), in_=xoh[:])

            # ===================== edges =====================
            with (
                tc.tile_pool(name="persist", bufs=1) as pp,
                tc.tile_pool(name="stream", bufs=2) as sp,
                tc.tile_pool(name="vec", bufs=2) as vp,
                tc.tile_pool(name="small", bufs=2) as mp,
                tc.tile_pool(name="psum", bufs=1, space="PSUM") as qp,
            ):
                ecl_t = [pp.tile([128, 4 * N], I32, name=f"ecl{g}", tag=f"ecl{g}")
                         for g in range(GPB)]
                # key tiles are reused as U (upper-tri value matrix) in phase D
                key_t = [pp.tile([128, 4 * N], F32, name=f"key{g}", tag=f"key{g}")
                         for g in range(GPB)]
                sel_t = [pp.tile([128, 4 * N], I8, name=f"sel{g}", tag=f"sel{g}")
                         for g in range(GPB)]
                cand = pp.tile([128, GPB, 256], F32)
                cntp = pp.tile([128, GPB], F32)

                # ---- phase A: keys + candidates + counts ----
                for g in range(GPB):
                    nc.sync.dma_start(
                        out=ecl_t[g][:].rearrange("p (t j) -> p t j", t=4),
                        in_=ecl[g].rearrange("(t p) j -> p t j", p=128))
                    gtile = sp.tile([128, 4 * N], F32, name="ged", tag="ged")
                    nc.sync.dma_start(
                        out=gtile[:].rearrange("p (t j) -> p t j", t=4),
                        in_=ged[g].rearrange("(t p) j -> p t j", p=128))
                    nc.vector.scalar_tensor_tensor(
                        out=key_t[g][:], in0=ecl_t[g][:], scalar=5.0, in1=gtile[:],
                        op0=AL.is_equal, op1=AL.mult)
                    nc.vector.tensor_add(key_t[g][:], key_t[g][:], negm[:])
                    scr = sp.tile([128, 4 * N], F32, name="scr", tag="ged")
                    nc.vector.tensor_scalar(
                        out=scr[:], in0=key_t[g][:], scalar1=0.0, scalar2=None,
                        op0=AL.is_gt)
                    nc.vector.tensor_reduce(cntp[:, g:g + 1], scr[:], axis=AX.X,
                                            op=AL.add)
                    for u in range(32):
                        nc.vector.max(cand[:, g, 8 * u:8 * u + 8],
                                      key_t[g][:, 64 * u:64 * (u + 1)])

                cnt_ps = qp.tile([128, GPB], F32, bufs=1)
                nc.tensor.matmul(cnt_ps[:], ones[:], cntp[:])
                ktile = mp.tile([128, GPB], F32, tag="ktile")
                nc.vector.tensor_scalar(out=ktile[:], in0=cnt_ps[:],
                                        scalar1=ke_bc[:, 0:1],
                                        scalar2=None, op0=AL.min)

                # ---- phase B: batched float bisection ----
                lo = mp.tile([128, GPB], F32, tag="lo")
                hi = mp.tile([128, GPB], F32, tag="hi")
                mid = mp.tile([128, GPB], F32, tag="mid")
                nc.vector.memset(lo[:], -6.0)
                nc.vector.memset(hi[:], 16.0)
                nc.vector.memset(mid[:], 5.0)
                for it in range(BISECT_ITERS):
                    gtm = vp.tile([128, GPB * 256], F32, tag="gtm")
                    mid_b = mid[:].rearrange("p (g o) -> p g o", o=1).broadcast_to([128, GPB, 256])
                    nc.vector.tensor_tensor(gtm[:].rearrange("p (g c) -> p g c", g=GPB),
                                            cand[:], mid_b, op=AL.is_gt)
                    cc = vp.tile([128, GPB], F32, tag="cc")
                    nc.vector.tensor_reduce(cc[:], gtm[:].rearrange("p (g c) -> p g c", g=GPB),
                                            axis=AX.X, op=AL.add)
                    cps = qp.tile([128, GPB], F32, tag="cps", bufs=2)
                    nc.tensor.matmul(cps[:], ones[:], cc[:])
                    gem = vp.tile([128, GPB], I8, tag="gem")
                    nc.vector.tensor_tensor(gem[:], cps[:], ktile[:], op=AL.is_ge)
                    nc.vector.copy_predicated(lo[:], gem[:], mid[:])
                    ltm = vp.tile([128, GPB], I8, tag="ltm")
                    nc.vector.tensor_scalar(out=ltm[:], in0=gem[:], scalar1=0.5,
                                            scalar2=None, op0=AL.is_lt)
                    nc.vector.copy_predicated(hi[:], ltm[:], mid[:])
                    nc.vector.tensor_add(mid[:], lo[:], hi[:])
                    nc.vector.tensor_scalar_mul(mid[:], mid[:], 0.5)

                # sel = key > lo  (exact: no representable value inside (lo,hi))
                for g in range(GPB):
                    nc.vector.tensor_scalar(out=sel_t[g][:], in0=key_t[g][:],
                                            scalar1=lo[:, g:g + 1], scalar2=None,
                                            op0=AL.is_gt)

                # ---- phase D: edge pred + U (key tiles overwritten as U) ----
                H = N // 2
                for g in range(GPB):
                    for t in range(4):
                        for h in range(2):
                            j0 = H * h
                            lxt = sp.tile([128, H * C6], F32, name="lxt", tag="lxt")
                            nc.sync.dma_start(
                                out=lxt[:],
                                in_=les[g, 128 * t:128 * (t + 1), j0:j0 + H]
                                .rearrange("p j c -> p (j c)"))
                            get_ = sp.tile([128, H * C6], F32, name="get", tag="get")
                            nc.scalar.dma_start(
                                out=get_[:],
                                in_=ge6[g, 128 * t:128 * (t + 1), j0:j0 + H]
                                .rearrange("p j c -> p (j c)"))
                            nc.vector.tensor_add(lxt[:], lxt[:], get_[:])
                            pv = vp.tile([128, H], F32, name="pv", tag="pv")
                            i6eb = (iota6f[:, :].rearrange("p (o c) -> p o c", o=1)
                                    .broadcast_to([128, H, 6]))
                            _argmax6(nc, vp, lxt[:].rearrange("p (j c) -> p j c", c=6),
                                     128, H, pv[:], "e", i6eb)
                            ecf = vp.tile([128, H], F32, name="ecf", tag="ecf")
                            nc.vector.tensor_copy(
                                ecf[:], ecl_t[g][:, N * t + j0:N * t + j0 + H])
                            nc.vector.copy_predicated(
                                ecf[:], sel_t[g][:, N * t + j0:N * t + j0 + H], pv[:])
                            nc.vector.tensor_mul(
                                key_t[g][:, N * t + j0:N * t + j0 + H], ecf[:],
                                tmask[:, N * t + j0:N * t + j0 + H])

                # ---- phase E: symmetrize + one-hot + store ----
                for g in range(GPB):
                    for s in range(4):
                        pst = qp.tile([128, N], F32, name="pst", tag="pst", bufs=4)
                        for t in range(4):
                            nc.tensor.transpose(
                                pst[:, 128 * t:128 * (t + 1)],
                                key_t[g][:, N * t + 128 * s:N * t + 128 * (s + 1)],
                                ident[:])
                        symv = sp.tile([128, N], F32, name="symv", tag="symv")
                        nc.vector.tensor_add(symv[:], key_t[g][:, N * s:N * (s + 1)],
                                             pst[:])
                        nc.gpsimd.tensor_add(symv[:], symv[:],
                                             dmask6[:, N * s:N * (s + 1)])
                        for h in range(2):
                            j0 = H * h
                            oh = sp.tile([128, H * C6], F32, name="oh", tag="oh")
                            symr = (symv[:, j0:j0 + H]
                                    .rearrange("p (j o) -> p j o", o=1)
                                    .broadcast_to([128, H, 6]))
                            i6e = (iota6f[:, :].rearrange("p (o c) -> p o c", o=1)
                                   .broadcast_to([128, H, 6]))
                            nc.vector.tensor_tensor(
                                oh[:].rearrange("p (j c) -> p j c", c=6),
                                symr, i6e, op=AL.is_equal)
                            nc.sync.dma_start(
                                out=enew[g, 128 * s:128 * (s + 1), j0:j0 + H]
                                .rearrange("p j c -> p (j c)"),
                                in_=oh[:])
    _split_multi_waits(nc)
    return nc


_NC = None


def make_in_maps(logits_x, logits_e, g_node, g_x, g_edge, g_e, Xclass, Eclass,
                 step_nodes, step_edges):
    B = Xclass.shape[0]
    ncore = 8
    gpc = B // ncore
    in_maps = []
    for c in range(ncore):
        s = slice(c * gpc, (c + 1) * gpc)
        in_maps.append({
            "eclass": np.ascontiguousarray(Eclass[s], dtype=np.int32),
            "gedge": np.ascontiguousarray(g_edge[s], dtype=np.float32),
            "logitse": np.ascontiguousarray(logits_e[s], dtype=np.float32),
            "ge6": np.ascontiguousarray(g_e[s], dtype=np.float32),
            "logitsx": np.ascontiguousarray(logits_x[s], dtype=np.float32),
            "gx6": np.ascontiguousarray(g_x[s], dtype=np.float32),
            "xclass": np.ascontiguousarray(Xclass[s], dtype=np.int32),
            "gnode": np.ascontiguousarray(g_node[s], dtype=np.float32),
            "stepe": np.full((128, 1), float(step_edges), dtype=np.float32),
            "stepn": np.full((128, 1), float(step_nodes), dtype=np.float32),
        })
    return in_maps


def kernel(logits_x, logits_e, g_node, g_x, g_edge, g_e, Xclass, Eclass,
           step_nodes, step_edges):
    global _NC
    ncore = 8
    assert Xclass.shape[0] // ncore == GPB
    if _NC is None:
        _NC = build_kernel()
    in_maps = make_in_maps(logits_x, logits_e, g_node, g_x, g_edge, g_e,
                           Xclass, Eclass, step_nodes, step_edges)
    res = run_bass_kernel_spmd(_NC, in_maps, core_ids=list(range(ncore)))
    Xnew = np.concatenate([res.results[c]["xnew"] for c in range(ncore)], axis=0)
    Enew = np.concatenate([res.results[c]["enew"] for c in range(ncore)], axis=0)
    return Xnew, Enew
